# revision 2
# baseline (speedup 1.0000x reference)
"""Batched teacher-forced attention decoder, data-parallel over 8 NeuronCores.

Key reformulation: the reference's sequential scan over graph_size is fully
parallelizable because the selections come from eval_seq (teacher forcing):
  * all step contexts are known upfront -> one batched query projection
  * the visited mask at step i is (first_visit_step[n] < i), computed once
    from eval_seq, so masked attention over all 512 steps is one batched
    masked softmax instead of 512 dependent steps.

Sharding: pure data parallel on the batch dim (B=128 -> 16 per core),
weights replicated, no collectives.
"""

import numpy as np
import jax
import jax.numpy as jnp

B, N, D, H = 128, 512, 128, 8
DK = D // H
CLIP = 10.0
NORM = float(np.sqrt(D))
NCORES = 8
BS = B // NCORES

_PREC = jax.lax.Precision.HIGHEST


def _shard_fn(emb, seq, fv, W_placeholder, W_node, W_fixed, W_step, W_out):
    # emb [BS,N,D] f32, seq [BS,N] i32, fv [BS,N] f32 (first-visit step, inf if never)
    proj = jnp.einsum("bnd,df->bnf", emb, W_node, precision=_PREC)
    gK, gV, lK = jnp.split(proj, 3, axis=-1)
    fixed = jnp.einsum("bd,de->be", emb.mean(axis=1), W_fixed, precision=_PREC)

    bix = jnp.arange(BS)
    emb_first = emb[bix, seq[:, 0]]                                   # [BS,D]
    emb_prev = jnp.take_along_axis(emb, seq[:, :-1, None], axis=1)    # [BS,N-1,D]
    ctx_rest = jnp.concatenate(
        [jnp.broadcast_to(emb_first[:, None], (BS, N - 1, D)), emb_prev], axis=-1
    )
    ctx0 = jnp.broadcast_to(W_placeholder[None, None], (BS, 1, 2 * D))
    ctx = jnp.concatenate([ctx0, ctx_rest], axis=1)                   # [BS,N,2D]

    Q = fixed[:, None] + jnp.einsum("bic,cd->bid", ctx, W_step, precision=_PREC)
    qh = Q.reshape(BS, N, H, DK)
    kh = gK.reshape(BS, N, H, DK)
    compat = NORM * jnp.einsum("bihd,bnhd->bhin", qh, kh, precision=_PREC)

    steps = jnp.arange(N, dtype=jnp.float32)
    mask = fv[:, None, :] < steps[None, :, None]                      # [BS,N(i),N(n)]
    compat = jnp.where(mask[:, None], -jnp.inf, compat)
    attn = jax.nn.softmax(compat, axis=-1)

    vh = gV.reshape(BS, N, H, DK)
    heads = jnp.einsum("bhin,bnhd->bihd", attn, vh, precision=_PREC).reshape(BS, N, D)
    glimpse = jnp.einsum("bid,de->bie", heads, W_out, precision=_PREC)
    logits = NORM * jnp.einsum("bnd,bid->bin", lK, glimpse, precision=_PREC)
    logits = jnp.tanh(logits) * CLIP
    logits = jnp.where(mask, -jnp.inf, logits)
    return jax.nn.log_softmax(logits, axis=-1)                        # [BS,N,N]


_pmapped = None


def _get_pmapped():
    global _pmapped
    if _pmapped is None:
        _pmapped = jax.pmap(
            _shard_fn, in_axes=(0, 0, 0, None, None, None, None, None)
        )
    return _pmapped


def _first_visit(seq):
    # fv[b,n] = first step index j with seq[b,j]==n, +inf if never selected.
    fv = np.full((B, N), np.inf, dtype=np.float32)
    js = np.tile(np.arange(N, dtype=np.float32), B)
    bb = np.repeat(np.arange(B), N)
    np.minimum.at(fv, (bb, seq.reshape(-1).astype(np.int64)), js)
    return fv


def kernel(embeddings, eval_seq, W_placeholder, W_node, W_fixed, W_step, W_out):
    emb = np.asarray(embeddings, dtype=np.float32)
    seq = np.asarray(eval_seq, dtype=np.int32)
    fv = _first_visit(seq)

    emb_s = emb.reshape(NCORES, BS, N, D)
    seq_s = seq.reshape(NCORES, BS, N)
    fv_s = fv.reshape(NCORES, BS, N)

    Wp = np.asarray(W_placeholder, np.float32)
    Wn = np.asarray(W_node, np.float32)
    Wf = np.asarray(W_fixed, np.float32)
    Ws = np.asarray(W_step, np.float32)
    Wo = np.asarray(W_out, np.float32)

    try:
        out = _get_pmapped()(emb_s, seq_s, fv_s, Wp, Wn, Wf, Ws, Wo)
        log_p = np.asarray(out).reshape(B, N, N)
    except Exception:
        # Fallback: per-device jit (async dispatch still overlaps devices),
        # and finally single-device if that fails too.
        try:
            devs = jax.devices()[:NCORES]
            fn = jax.jit(_shard_fn)
            futs = []
            for c, dev in enumerate(devs):
                args = [jax.device_put(a, dev) for a in
                        (emb_s[c], seq_s[c], fv_s[c], Wp, Wn, Wf, Ws, Wo)]
                futs.append(fn(*args))
            log_p = np.concatenate([np.asarray(f) for f in futs], 0).reshape(B, N, N)
        except Exception:
            fn = jax.jit(_shard_fn)
            parts = [np.asarray(fn(emb_s[c], seq_s[c], fv_s[c], Wp, Wn, Wf, Ws, Wo))
                     for c in range(NCORES)]
            log_p = np.concatenate(parts, 0).reshape(B, N, N)

    return log_p.astype(np.float32), np.asarray(eval_seq)


# revision 6
# speedup vs baseline: 1.3555x; 1.3555x over previous
"""Batched teacher-forced attention decoder, data-parallel over 8 NeuronCores.

Key reformulation: the reference's sequential scan over graph_size is fully
parallelizable because the selections come from eval_seq (teacher forcing):
  * all step contexts are known upfront -> one batched query projection
  * the visited mask at step i is (first_visit_step[n] < i), computed once
    from eval_seq, so masked attention over all 512 steps is one batched
    masked softmax instead of 512 dependent steps.

Sharding: pure data parallel on the batch dim (B=128 -> 16 per core),
weights replicated, no collectives.
"""

import numpy as np
import jax
import jax.numpy as jnp

B, N, D, H = 128, 512, 128, 8
DK = D // H
CLIP = 10.0
NORM = float(np.sqrt(D))
NCORES = 8
BS = B // NCORES

_PREC = jax.lax.Precision.HIGHEST


def _shard_fn(emb, seq, fv, W_placeholder, W_node, W_fixed, W_step, W_out):
    # emb [BS,N,D] f32, seq [BS,N] i32, fv [BS,N] f32 (first-visit step, inf if never)
    proj = jnp.einsum("bnd,df->bnf", emb, W_node, precision=_PREC)
    gK, gV, lK = jnp.split(proj, 3, axis=-1)
    fixed = jnp.einsum("bd,de->be", emb.mean(axis=1), W_fixed, precision=_PREC)

    bix = jnp.arange(BS)
    emb_first = emb[bix, seq[:, 0]]                                   # [BS,D]
    emb_prev = jnp.take_along_axis(emb, seq[:, :-1, None], axis=1)    # [BS,N-1,D]
    ctx_rest = jnp.concatenate(
        [jnp.broadcast_to(emb_first[:, None], (BS, N - 1, D)), emb_prev], axis=-1
    )
    ctx0 = jnp.broadcast_to(W_placeholder[None, None], (BS, 1, 2 * D))
    ctx = jnp.concatenate([ctx0, ctx_rest], axis=1)                   # [BS,N,2D]

    Q = fixed[:, None] + jnp.einsum("bic,cd->bid", ctx, W_step, precision=_PREC)
    qh = Q.reshape(BS, N, H, DK)
    kh = gK.reshape(BS, N, H, DK)
    compat = NORM * jnp.einsum("bihd,bnhd->bhin", qh, kh, precision=_PREC)

    steps = jnp.arange(N, dtype=jnp.float32)
    mask = fv[:, None, :] < steps[None, :, None]                      # [BS,N(i),N(n)]
    # Use a large-finite negative instead of -inf on device: exp((-1e30)-max)
    # underflows to exactly 0, giving identical softmax math with no inf/NaN
    # hazards in the neuron backend. Exact -inf is stamped host-side.
    compat = jnp.where(mask[:, None], -1e30, compat)
    attn = jax.nn.softmax(compat, axis=-1)

    vh = gV.reshape(BS, N, H, DK)
    heads = jnp.einsum("bhin,bnhd->bihd", attn, vh, precision=_PREC).reshape(BS, N, D)
    glimpse = jnp.einsum("bid,de->bie", heads, W_out, precision=_PREC)
    logits = NORM * jnp.einsum("bnd,bid->bin", lK, glimpse, precision=_PREC)
    logits = jnp.tanh(logits) * CLIP
    logits = jnp.where(mask, -1e30, logits)
    return jax.nn.log_softmax(logits, axis=-1)                        # [BS,N,N]


_pmapped = None


def _get_pmapped():
    global _pmapped
    if _pmapped is None:
        _pmapped = jax.pmap(
            _shard_fn, in_axes=(0, 0, 0, None, None, None, None, None)
        )
    return _pmapped


def _first_visit(seq):
    # fv[b,n] = first step index j with seq[b,j]==n, 1e9 if never selected.
    fv = np.full((B, N), 1e9, dtype=np.float32)
    js = np.tile(np.arange(N, dtype=np.float32), B)
    bb = np.repeat(np.arange(B), N)
    np.minimum.at(fv, (bb, seq.reshape(-1).astype(np.int64)), js)
    return fv


def kernel(embeddings, eval_seq, W_placeholder, W_node, W_fixed, W_step, W_out):
    emb = np.asarray(embeddings, dtype=np.float32)
    seq = np.asarray(eval_seq, dtype=np.int32)
    fv = _first_visit(seq)

    emb_s = emb.reshape(NCORES, BS, N, D)
    seq_s = seq.reshape(NCORES, BS, N)
    fv_s = fv.reshape(NCORES, BS, N)

    Wp = np.asarray(W_placeholder, np.float32)
    Wn = np.asarray(W_node, np.float32)
    Wf = np.asarray(W_fixed, np.float32)
    Ws = np.asarray(W_step, np.float32)
    Wo = np.asarray(W_out, np.float32)

    try:
        out = _get_pmapped()(emb_s, seq_s, fv_s, Wp, Wn, Wf, Ws, Wo)
        log_p = np.asarray(out).reshape(B, N, N)
    except Exception:
        # Fallback: per-device jit (async dispatch still overlaps devices),
        # and finally single-device if that fails too.
        try:
            devs = jax.devices()[:NCORES]
            fn = jax.jit(_shard_fn)
            futs = []
            for c, dev in enumerate(devs):
                args = [jax.device_put(a, dev) for a in
                        (emb_s[c], seq_s[c], fv_s[c], Wp, Wn, Wf, Ws, Wo)]
                futs.append(fn(*args))
            log_p = np.concatenate([np.asarray(f) for f in futs], 0).reshape(B, N, N)
        except Exception:
            fn = jax.jit(_shard_fn)
            parts = [np.asarray(fn(emb_s[c], seq_s[c], fv_s[c], Wp, Wn, Wf, Ws, Wo))
                     for c in range(NCORES)]
            log_p = np.concatenate(parts, 0).reshape(B, N, N)

    # Stamp exact -inf at masked positions (mask[b,i,n] = fv[b,n] < i),
    # matching the reference's masked log-probabilities bit-for-bit.
    log_p = log_p.astype(np.float32)
    steps = np.arange(N, dtype=np.float32)
    host_mask = fv[:, None, :] < steps[None, :, None]
    log_p[host_mask] = -np.inf

    return log_p, np.asarray(eval_seq)
